# revision 76
# baseline (speedup 1.0000x reference)
"""Multi-head attention (RoPE, causal) Trainium2 Bass kernel, 8-core SPMD.

Sharding: tensor-parallel over heads (2 heads/core) for QKV+attention;
the output projection is contraction-parallel: each core multiplies its
head-pair's attention output by its 128-row slice of Wo and streams a
full-token partial y to DRAM; the 8-way reduction happens on the host.
This removes the AllToAll entirely (its cost model is 15us overhead +
2MB/40GBps ~= 67us of serial tail).

bf16 activations/weights throughout (inputs host-cast); PSUM stays f32.

Math per core c (heads h0=2c, h1=2c+1):
  qT = Wq2.T @ xT            (feature-on-partition layout throughout)
  rope: qrot = qT*cosT + P2@(qT*sinT)   (P2 = pair-swap-with-sign, const)
  scoresT[k,q] = krot.T @ qrot tiles    ([k-part, q-free] layout)
  attnT = exp(scoresT/8), causal-masked on diagonal tiles
  AV: out[65,q] = [v | ones].T @ attnT  (row 64 = softmax denominator)
  normalize, po[tok,1024] = stage.T @ Wo_c, stream to y_partial DRAM
"""
import os
import numpy as np
import ml_dtypes
from contextlib import ExitStack

import concourse.bass as bass
import concourse.mybir as mybir
import concourse.tile as tile
from concourse import library_config

_SIM = os.environ.get("BASS_SIM") == "1"   # CoreSim (CPU) iteration mode

N_CORES = 8
B, S, D, H, DK = 2, 2048, 1024, 16, 64
T = B * S                    # 4096 flat tokens, batch-major
TT = 512                     # token tile (phase 1 / q tiles)
KT = 128                     # k tile (scores partition dim)
NT = T // TT                 # 8 token tiles
F32 = mybir.dt.float32
BF16 = mybir.dt.bfloat16
F8E4 = mybir.dt.float8e4          # e4m3: v / x / wv (max 240)
F8E5 = mybir.dt.float8e5          # e5m2: exp(scores) (wide range)
AF = mybir.ActivationFunctionType
PM = mybir.MatmulPerfMode
SCALE = 1.0 / np.sqrt(DK)
EXPB = -3.0                       # exp bias, cancelled by the denominator
NPBF = ml_dtypes.bfloat16
NPF8 = ml_dtypes.float8_e4m3
VP = 72                           # v_sb per-head stride (64 v + 1 ones + pad)

_cache = {}


def _consts():
    inv_freq = 10000.0 ** (-(np.arange(0, DK, 2, dtype=np.float64) / DK))
    pos = np.arange(S, dtype=np.float64)
    ang = pos[:, None] * inv_freq[None, :]                 # [S, 32]
    cos = np.repeat(np.cos(ang), 2, axis=1).T              # [64, S]
    sin = np.repeat(np.sin(ang), 2, axis=1).T
    cosT = np.concatenate([cos, cos], 0).astype(NPBF)   # [128, S]
    sinT = np.concatenate([sin, sin], 0).astype(NPBF)
    # P2T = P.T blockdiag for 2 heads; (P v)[2i] = -v[2i+1], (P v)[2i+1] = v[2i]
    p = np.zeros((DK, DK), np.float32)
    for i in range(DK // 2):
        p[2 * i, 2 * i + 1] = -1.0
        p[2 * i + 1, 2 * i] = 1.0
    p2t = np.zeros((128, 128), np.float32)
    p2t[:DK, :DK] = p.T
    p2t[DK:, DK:] = p.T
    ident = np.eye(128, dtype=np.float32)
    ones64 = np.ones((1, DK), np.float32)
    return (cosT, sinT, p2t.astype(NPBF), ident.astype(NPBF),
            ones64.astype(NPBF))


def split_multi_waits(nc, max_waits=1):
    """This walrus build allows fewer sync-waits per instruction than Tile's
    final drain carries; hoist extras onto same-engine NOPs inserted before."""
    for fn in nc.m.functions:
        for blk in fn.blocks:
            insts = blk.instructions
            out = []
            for inst in insts:
                si = getattr(inst, "sync_info", None)
                waits = list(si.on_wait) if si is not None else []
                if len(waits) > max_waits:
                    extra, keep = waits[:-max_waits], waits[-max_waits:]
                    for j, w in enumerate(extra):
                        nop = mybir.InstNoOp(
                            name=f"{inst.name}-wsplit{j}", ins=[], outs=[]
                        )
                        nop.engine = inst.engine
                        nop.sync_info = mybir.SyncInfo(on_wait=[w], on_update=[])
                        out.append(nop)
                    inst.sync_info = mybir.SyncInfo(
                        on_wait=keep, on_update=list(si.on_update)
                    )
                out.append(inst)
            insts[:] = out


def build_nc(repeat=1):
    cosT_np, sinT_np, p2t_np, ident_np, ones64_np = _consts()

    nc = bass.Bass("TRN2", target_bir_lowering=False, debug=False,
                   num_devices=N_CORES)
    xT = nc.declare_dram_parameter("xT", [D, T], BF16, isOutput=False)
    wq = nc.declare_dram_parameter("wq", [D, 128], BF16, isOutput=False)
    wk = nc.declare_dram_parameter("wk", [D, 128], BF16, isOutput=False)
    wv = nc.declare_dram_parameter("wv", [D, 128], BF16, isOutput=False)
    wo = nc.declare_dram_parameter("wo", [128, D], BF16, isOutput=False)
    y = nc.declare_dram_parameter("y", [T, D], BF16, isOutput=True)

    dnorm = nc.dram_tensor("dnorm", [2, 2, 1, TT], BF16)   # 1/denom bounce
    c_cos = nc.inline_tensor(cosT_np, name="c_cos")
    c_sin = nc.inline_tensor(sinT_np, name="c_sin")
    c_p2t = nc.inline_tensor(p2t_np, name="c_p2t")
    c_id = nc.inline_tensor(ident_np, name="c_id")
    c_on = nc.inline_tensor(ones64_np, name="c_on")

    with tile.TileContext(nc) as tc, ExitStack() as ctx:
        cst = ctx.enter_context(tc.tile_pool(name="cst", bufs=1))
        stream = ctx.enter_context(tc.tile_pool(name="stream", bufs=2))
        persist = ctx.enter_context(tc.tile_pool(name="persist", bufs=1))
        tmp = ctx.enter_context(tc.tile_pool(name="tmp", bufs=3))
        attnp = ctx.enter_context(tc.tile_pool(name="attnp", bufs=4))
        outp = ctx.enter_context(tc.tile_pool(name="outp", bufs=4))
        ps = ctx.enter_context(tc.tile_pool(name="ps", bufs=2, space="PSUM"))
        psav = ctx.enter_context(tc.tile_pool(name="psav", bufs=2, space="PSUM"))

        # ---- constants + weights to SBUF (ordered by first use) ----
        cos_s = cst.tile([128, S], BF16)
        sin_s = cst.tile([128, S], BF16)
        p2t_s = cst.tile([128, 128], BF16)
        id_s = cst.tile([128, 128], BF16)
        on_s = cst.tile([1, DK], BF16)
        wq_s = cst.tile([128, 8, 128], BF16)
        wk_s = cst.tile([128, 8, 128], BF16)
        wv_s = cst.tile([128, 8, 128], BF16)
        wo_s = cst.tile([128, D], BF16)
        def issue_xt(t):
            """Prefetch token tile t of x (feature-major)."""
            xt = stream.tile([128, 8, TT], BF16, tag="xt")
            for g in range(8):
                nc.sync.dma_start(
                    out=xt[:, g, :],
                    in_=xT[g * 128:(g + 1) * 128, t * TT:(t + 1) * TT],
                )
            return xt

        xt0 = issue_xt(0)          # before the const loads on the sync queue
        # first-needed consts on the scalar queue, the rest trail xt0 on sync
        nc.scalar.dma_start(
            out=wq_s[:], in_=wq.ap().rearrange("(g p) m -> p g m", p=128))
        nc.scalar.dma_start(out=sin_s[:], in_=c_sin[:, :])
        nc.scalar.dma_start(out=cos_s[:], in_=c_cos[:, :])
        nc.scalar.dma_start(out=p2t_s[:], in_=c_p2t[:, :])
        nc.sync.dma_start(
            out=wk_s[:], in_=wk.ap().rearrange("(g p) m -> p g m", p=128))
        nc.sync.dma_start(
            out=wv_s[:], in_=wv.ap().rearrange("(g p) m -> p g m", p=128))
        nc.sync.dma_start(out=id_s[:], in_=c_id[:, :])
        nc.sync.dma_start(out=on_s[:], in_=c_on[:, :])
        nc.sync.dma_start(out=wo_s[:], in_=wo.ap())

        # persistent activations
        qrot = persist.tile([128, T], BF16)
        krot = persist.tile([128, T], BF16)
        # per k-block: [v_h0(64) | ones | pad to 72 | v_h1(64) | ones | pad]
        v_sb = persist.tile([128, T // KT, 2 * VP], BF16)
        # cols 64/136 stay 1.0 (denominator ones)
        nc.vector.memset(v_sb[:].rearrange("p a b -> p (a b)"), 1.0)

        def phase1_qk(t, xt):
            """Project token tile t to q/k and rope them."""
            pos = slice((t % (S // TT)) * TT, (t % (S // TT)) * TT + TT)
            tok = slice(t * TT, (t + 1) * TT)
            for w_sb, dst in ((wq_s, qrot), (wk_s, krot)):
                p_q = ps.tile([128, TT], F32, tag="p1")
                for g in range(8):
                    nc.tensor.matmul(p_q[:], w_sb[:, g, :], xt[:, g, :],
                                     start=(g == 0), stop=(g == 7))
                qs = tmp.tile([128, TT], BF16, tag="qs")
                nc.vector.tensor_mul(qs[:], p_q[:], sin_s[:, pos])
                p_perm = ps.tile([128, TT], F32, tag="p1")
                nc.tensor.matmul(p_perm[:], p2t_s[:], qs[:], start=True, stop=True)
                qc = tmp.tile([128, TT], F32, tag="qc")
                nc.vector.tensor_mul(qc[:], p_q[:], cos_s[:, pos])
                nc.vector.tensor_add(dst[:, tok], qc[:], p_perm[:])

        def phase1_v(t, xt):
            """Project v for tile t and transpose to [k-pos, dk] layout."""
            p_v = ps.tile([128, TT], F32, tag="p1")
            for g in range(8):
                nc.tensor.matmul(p_v[:], wv_s[:, g, :], xt[:, g, :],
                                 start=(g == 0), stop=(g == 7))
            vt = tmp.tile([128, TT], BF16, tag="vt")
            nc.vector.tensor_copy(vt[:], p_v[:])
            for blk in range(TT // 128):
                p_t = ps.tile([128, 128], BF16, tag="p1")
                nc.tensor.transpose(p_t[:], vt[:, blk * 128:(blk + 1) * 128], id_s[:])
                g = t * (TT // 128) + blk
                vdst = v_sb[:, g, :].rearrange("p (a c) -> p a c", a=2)[:, :, 0:64]
                nc.vector.tensor_copy(
                    vdst, p_t[:].rearrange("p (a c) -> p a c", a=2)
                )

        def make_attention(b, J):
            """q-tile J (512 wide) of batch b, both heads paired.

            Returns (scores_exp, av, finish) emitters so the caller can
            interleave score/exp of early blocks with phase1_v and keep a
            2-block exp lookahead ahead of the AV accumulation.
            """
            av0 = psav.tile([65, TT], F32, tag="av")
            av1 = psav.tile([65, TT], F32, tag="av")
            av = [av0, av1]
            nk = 4 * (J + 1)
            ats = {}

            def scores_exp(i):
                r = i - 4 * J          # >= 0 on diagonal blocks
                qo = KT * r if r > 0 else 0    # causal-narrowed q offset
                n = TT - qo
                p_s = ps.tile([128, 2, TT], F32, tag="mm")
                for h in range(2):
                    hp = slice(64 * h, 64 * h + 64)
                    nc.tensor.matmul(
                        p_s[:, h, 0:n],
                        krot[hp, b * S + i * KT: b * S + (i + 1) * KT],
                        qrot[hp, b * S + J * TT + qo: b * S + (J + 1) * TT],
                        start=True, stop=True,
                    )
                at = attnp.tile([128, 2, TT], BF16, tag="at")
                nc.scalar.activation(at[:, :, 0:n], p_s[:, :, 0:n], AF.Exp,
                                     scale=float(SCALE))
                if r >= 0:  # diagonal 128-block: zero where k > q
                    for h in range(2):
                        nc.gpsimd.affine_select(
                            out=at[:, h, 0:KT], in_=at[:, h, 0:KT],
                            compare_op=mybir.AluOpType.is_ge,
                            fill=0.0, base=0,
                            pattern=[[1, KT]], channel_multiplier=-1,
                        )
                ats[i] = at

            def av_acc(i):
                r = i - 4 * J
                qo = KT * r if r > 0 else 0
                n = TT - qo
                at = ats.pop(i)
                g = b * (S // KT) + i
                for h in range(2):
                    nc.tensor.matmul(
                        av[h][:, qo:TT],
                        v_sb[:, g, VP * h:VP * h + 65],
                        at[:, h, 0:n],
                        start=(i == 0), stop=(i == nk - 1),
                    )

            stage = outp.tile([128, TT], BF16, tag="stage")

            def finish_norm(cs=slice(0, TT)):
                # 1/denom on DVE straight from PSUM, replicate across 64
                # partitions with a ones-column matmul, then scale av.
                w = cs.stop - cs.start
                recs = []
                for h in range(2):
                    rec = tmp.tile([1, TT], BF16, tag="rec")
                    with nc.allow_low_precision(reason="bf16 recip"):
                        nc.vector.reciprocal(rec[:, 0:w], av[h][64:65, cs])
                    recs.append(rec)
                pbs = []
                for h in range(2):
                    pb = ps.tile([64, TT], F32, tag="p1")
                    nc.tensor.matmul(pb[:, 0:w], on_s[:], recs[h][:, 0:w],
                                     start=True, stop=True)
                    pbs.append(pb)
                for h in range(2):  # PSUM->SBUF hop (hw: one PSUM input/op)
                    hp = slice(64 * h, 64 * h + 64)
                    if h == 0:
                        nc.scalar.copy(stage[hp, cs], av[h][0:64, cs])
                    else:
                        nc.vector.tensor_copy(stage[hp, cs], av[h][0:64, cs])
                for h in range(2):
                    hp = slice(64 * h, 64 * h + 64)
                    nc.vector.tensor_mul(stage[hp, cs], stage[hp, cs],
                                         pbs[h][:, 0:w])

            def finish_proj(blks=range(TT // 128)):
                # output projection for this tile: po[tok,1024] = stage.T@Wo_c
                # PSUM->SBUF copies go mostly to Act when the NEXT tile's
                # attention is small (Act has slack there), else mostly DVE.
                t = 4 * b + J
                act_heavy = None  # 2/2 split below
                for blk in blks:
                    po = ps.tile([128, 2, TT], F32, tag="mm")
                    for hf in range(2):  # psum bank = 512 f32: 2 half-matmuls
                        nc.tensor.matmul(po[:, hf, :],
                                         stage[:, blk * 128:(blk + 1) * 128],
                                         wo_s[:, hf * TT:(hf + 1) * TT],
                                         start=True, stop=True)
                    yt = outp.tile([128, D], BF16, tag="yt")
                    pov = po[:].rearrange("p a b -> p (a b)")
                    on_act = blk % 2 == 0
                    if on_act:
                        nc.scalar.copy(yt[:], pov)
                    else:
                        nc.vector.tensor_copy(yt[:], pov)
                    deng = nc.sync if blk % 2 == 0 else nc.gpsimd
                    deng.dma_start(
                        out=y[t * TT + blk * 128: t * TT + (blk + 1) * 128, :],
                        in_=yt[:],
                    )

            return nk, scores_exp, av_acc, finish_norm, finish_proj

        # interleave: attention(b, J) depends only on token tiles <= t;
        # exp of blocks 0/1 cooks during phase1_v; xt(t+1) prefetches during
        # attention(t); AV trails scores/exp by 2 blocks. The previous tile's
        # normalize+outproj are deferred into the next tile's phase1 so PE
        # has projection work to chew while DVE computes reciprocals.
        for rep in range(repeat):
            xt = xt0 if rep == 0 else issue_xt(0)
            fin = None
            for t in range(8):
                phase1_qk(t, xt)
                if fin is not None:
                    fin[0]()           # norm(t-1): recip/bcast/mul
                nk, scores_exp, av_acc, f_norm, f_proj = \
                    make_attention(t // 4, t % 4)
                scores_exp(0)
                scores_exp(1)
                phase1_v(t, xt)
                if t < 7:
                    xt = issue_xt(t + 1)
                if fin is not None:
                    fin[1]()           # outproj(t-1)
                for i in range(2, nk):
                    scores_exp(i)
                    av_acc(i - 2)
                av_acc(nk - 2)
                av_acc(nk - 1)
                fin = (f_norm, f_proj)
            # last tile: column-split finish so the first po matmuls start
            # after only half the normalize chain
            fin[0](slice(0, TT // 2))
            fin[1](range(0, 2))
            fin[0](slice(TT // 2, TT))
            fin[1](range(2, 4))

    if not _SIM:            # hw encoding workaround; confuses sim race detector
        split_multi_waits(nc)
    return nc


def _get_runner(repeat=1):
    """Build + jit once; returns f(in_maps) -> list of per-core output dicts."""
    key = ("runner", repeat)
    if key in _cache:
        return _cache[key]
    import jax
    import jax.numpy as jnp
    from jax.sharding import Mesh, PartitionSpec
    from jax.experimental.shard_map import shard_map
    from concourse import bass2jax, mybir as _mybir

    nc = build_nc(repeat=repeat)
    bass2jax.install_neuronx_cc_hook()

    in_names, out_names, out_avals, zero_outs = [], [], [], []
    for alloc in nc.m.functions[0].allocations:
        if not isinstance(_mybir.MemoryLocationSet, type) or not isinstance(
            alloc, _mybir.MemoryLocationSet
        ):
            continue
        name = alloc.memorylocations[0].name
        if alloc.kind == "ExternalInput":
            if name != "partition_id":
                in_names.append(name)
        elif alloc.kind == "ExternalOutput":
            out_names.append(name)
            shape = tuple(alloc.tensor_shape)
            dtype = _mybir.dt.np(alloc.dtype)
            out_avals.append(jax.core.ShapedArray(shape, dtype))
            zero_outs.append(np.zeros(shape, dtype))
    n_params = len(in_names)
    has_pid = nc.partition_id_tensor is not None
    all_names = in_names + out_names + (["partition_id"] if has_pid else [])

    def _body(*args):
        operands = list(args)
        if has_pid:
            operands.append(bass2jax.partition_id_tensor())
        outs = bass2jax._bass_exec_p.bind(
            *operands,
            out_avals=tuple(out_avals),
            in_names=tuple(all_names),
            out_names=tuple(out_names),
            lowering_input_output_aliases=(),
            sim_require_finite=True,
            sim_require_nnan=True,
            nc=nc,
        )
        return tuple(outs)

    devices = jax.devices()[:N_CORES]
    mesh = Mesh(np.asarray(devices), ("core",))
    n_outs = len(out_names)
    sharded = jax.jit(
        shard_map(
            _body, mesh=mesh,
            in_specs=(PartitionSpec("core"),) * (n_params + n_outs),
            out_specs=(PartitionSpec("core"),) * n_outs,
            check_rep=False,
        ),
        donate_argnums=() if _SIM else tuple(range(n_params, n_params + n_outs)),
        keep_unused=True,
    )

    def make_bench(in_maps):
        from jax.sharding import NamedSharding
        sh = NamedSharding(mesh, PartitionSpec("core"))
        concat_in = [
            jax.device_put(
                np.concatenate([np.asarray(m[nm]) for m in in_maps], axis=0), sh)
            for nm in in_names
        ]
        zshapes = [(N_CORES * z.shape[0], *z.shape[1:]) for z in zero_outs]
        zdt = [z.dtype for z in zero_outs]
        mkz = jax.jit(
            lambda: tuple(jnp.zeros(s, d) for s, d in zip(zshapes, zdt)),
            out_shardings=tuple(sh for _ in zshapes),
        )

        def bench_once():
            zs = mkz()
            jax.block_until_ready(zs)
            t0 = __import__("time").perf_counter()
            out = sharded(*concat_in, *zs)
            jax.block_until_ready(out)
            return __import__("time").perf_counter() - t0

        return bench_once

    def run(in_maps):
        concat_in = [
            np.concatenate([np.asarray(m[nm]) for m in in_maps], axis=0)
            for nm in in_names
        ]
        concat_zeros = [
            np.zeros((N_CORES * z.shape[0], *z.shape[1:]), z.dtype)
            for z in zero_outs
        ]
        out_arrs = sharded(*concat_in, *concat_zeros)
        return [
            {nm: np.asarray(out_arrs[i]).reshape(N_CORES, *out_avals[i].shape)[c]
             for i, nm in enumerate(out_names)}
            for c in range(N_CORES)
        ]

    run.make_bench = make_bench
    _cache[key] = run
    return run


def _prep_in_maps(x, Wq, Wk, Wv, Wo):
    xT = np.ascontiguousarray(x.reshape(T, D).T).astype(NPBF)
    in_maps = []
    for c in range(N_CORES):
        in_maps.append({
            "xT": xT,
            "wq": np.ascontiguousarray(
                np.concatenate([Wq[2 * c], Wq[2 * c + 1]], 1)).astype(NPBF),
            "wk": np.ascontiguousarray(
                np.concatenate([Wk[2 * c], Wk[2 * c + 1]], 1)).astype(NPBF),
            "wv": np.ascontiguousarray(
                np.concatenate([Wv[2 * c], Wv[2 * c + 1]], 1)).astype(NPBF),
            "wo": np.ascontiguousarray(Wo[128 * c:128 * (c + 1), :]).astype(NPBF),
        })
    return in_maps


def kernel(x, Wq, Wk, Wv, Wo, repeat=1):
    x, Wq, Wk, Wv, Wo = (np.asarray(a, np.float32) for a in (x, Wq, Wk, Wv, Wo))
    run = _get_runner(repeat=repeat)
    results = run(_prep_in_maps(x, Wq, Wk, Wv, Wo))
    out = results[0]["y"].astype(np.float32)
    for r in results[1:]:
        out += r["y"].astype(np.float32)
    return out.reshape(B, S, D)


# revision 82
# speedup vs baseline: 1.0418x; 1.0418x over previous
"""Multi-head attention (RoPE, causal) Trainium2 Bass kernel, 8-core SPMD.

Sharding: tensor-parallel over heads (2 heads/core) for QKV+attention;
the output projection is contraction-parallel: each core multiplies its
head-pair's attention output by its 128-row slice of Wo and streams a
full-token partial y to DRAM; the 8-way reduction happens on the host.
This removes the AllToAll entirely (its cost model is 15us overhead +
2MB/40GBps ~= 67us of serial tail).

bf16 activations/weights throughout (inputs host-cast); PSUM stays f32.

Math per core c (heads h0=2c, h1=2c+1):
  qT = Wq2.T @ xT            (feature-on-partition layout throughout)
  rope: qrot = qT*cosT + P2@(qT*sinT)   (P2 = pair-swap-with-sign, const)
  scoresT[k,q] = krot.T @ qrot tiles    ([k-part, q-free] layout)
  attnT = exp(scoresT/8), causal-masked on diagonal tiles
  AV: out[65,q] = [v | ones].T @ attnT  (row 64 = softmax denominator)
  normalize, po[tok,1024] = stage.T @ Wo_c, stream to y_partial DRAM
"""
import os
import numpy as np
import ml_dtypes
from contextlib import ExitStack

import concourse.bass as bass
import concourse.mybir as mybir
import concourse.tile as tile
from concourse import library_config

_SIM = os.environ.get("BASS_SIM") == "1"   # CoreSim (CPU) iteration mode

N_CORES = 8
B, S, D, H, DK = 2, 2048, 1024, 16, 64
T = B * S                    # 4096 flat tokens, batch-major
TT = 512                     # token tile (phase 1 / q tiles)
KT = 128                     # k tile (scores partition dim)
NT = T // TT                 # 8 token tiles
F32 = mybir.dt.float32
BF16 = mybir.dt.bfloat16
F8E4 = mybir.dt.float8e4          # e4m3: v / x / wv (max 240)
F8E5 = mybir.dt.float8e5          # e5m2: exp(scores) (wide range)
AF = mybir.ActivationFunctionType
PM = mybir.MatmulPerfMode
SCALE = 1.0 / np.sqrt(DK)
EXPB = -3.0                       # exp bias, cancelled by the denominator
NPBF = ml_dtypes.bfloat16
NPF8 = ml_dtypes.float8_e4m3
VP = 72                           # v_sb per-head stride (64 v + 1 ones + pad)

_cache = {}


def _consts():
    inv_freq = 10000.0 ** (-(np.arange(0, DK, 2, dtype=np.float64) / DK))
    pos = np.arange(S, dtype=np.float64)
    ang = pos[:, None] * inv_freq[None, :]                 # [S, 32]
    cos = np.repeat(np.cos(ang), 2, axis=1).T              # [64, S]
    sin = np.repeat(np.sin(ang), 2, axis=1).T
    cosT = np.concatenate([cos, cos], 0).astype(NPBF)   # [128, S]
    sinT = np.concatenate([sin, sin], 0).astype(NPBF)
    # P2T = P.T blockdiag for 2 heads; (P v)[2i] = -v[2i+1], (P v)[2i+1] = v[2i]
    p = np.zeros((DK, DK), np.float32)
    for i in range(DK // 2):
        p[2 * i, 2 * i + 1] = -1.0
        p[2 * i + 1, 2 * i] = 1.0
    p2t = np.zeros((128, 128), np.float32)
    p2t[:DK, :DK] = p.T
    p2t[DK:, DK:] = p.T
    ident = np.eye(128, dtype=np.float32)
    ones64 = np.ones((1, DK), np.float32)
    return (cosT, sinT, p2t.astype(NPBF), ident.astype(NPBF),
            ones64.astype(NPBF))


def split_multi_waits(nc, max_waits=1):
    """This walrus build allows fewer sync-waits per instruction than Tile's
    final drain carries; hoist extras onto same-engine NOPs inserted before."""
    for fn in nc.m.functions:
        for blk in fn.blocks:
            insts = blk.instructions
            out = []
            for inst in insts:
                si = getattr(inst, "sync_info", None)
                waits = list(si.on_wait) if si is not None else []
                if len(waits) > max_waits:
                    extra, keep = waits[:-max_waits], waits[-max_waits:]
                    for j, w in enumerate(extra):
                        nop = mybir.InstNoOp(
                            name=f"{inst.name}-wsplit{j}", ins=[], outs=[]
                        )
                        nop.engine = inst.engine
                        nop.sync_info = mybir.SyncInfo(on_wait=[w], on_update=[])
                        out.append(nop)
                    inst.sync_info = mybir.SyncInfo(
                        on_wait=keep, on_update=list(si.on_update)
                    )
                out.append(inst)
            insts[:] = out


def build_nc(repeat=1):
    cosT_np, sinT_np, p2t_np, ident_np, ones64_np = _consts()

    nc = bass.Bass("TRN2", target_bir_lowering=False, debug=False,
                   num_devices=N_CORES)
    xT = nc.declare_dram_parameter("xT", [D, T], BF16, isOutput=False)
    wq = nc.declare_dram_parameter("wq", [D, 128], BF16, isOutput=False)
    wk = nc.declare_dram_parameter("wk", [D, 128], BF16, isOutput=False)
    wv = nc.declare_dram_parameter("wv", [D, 128], BF16, isOutput=False)
    wo = nc.declare_dram_parameter("wo", [128, D], BF16, isOutput=False)
    y = nc.declare_dram_parameter("y", [T, D], BF16, isOutput=True)

    dnorm = nc.dram_tensor("dnorm", [2, 2, 1, TT], BF16)   # 1/denom bounce
    c_cos = nc.inline_tensor(cosT_np, name="c_cos")
    c_sin = nc.inline_tensor(sinT_np, name="c_sin")
    c_p2t = nc.inline_tensor(p2t_np, name="c_p2t")
    c_id = nc.inline_tensor(ident_np, name="c_id")
    c_on = nc.inline_tensor(ones64_np, name="c_on")

    with tile.TileContext(nc) as tc, ExitStack() as ctx:
        cst = ctx.enter_context(tc.tile_pool(name="cst", bufs=1))
        stream = ctx.enter_context(tc.tile_pool(name="stream", bufs=2))
        persist = ctx.enter_context(tc.tile_pool(name="persist", bufs=1))
        tmp = ctx.enter_context(tc.tile_pool(name="tmp", bufs=3))
        attnp = ctx.enter_context(tc.tile_pool(name="attnp", bufs=4))
        outp = ctx.enter_context(tc.tile_pool(name="outp", bufs=4))
        ps = ctx.enter_context(tc.tile_pool(name="ps", bufs=2, space="PSUM"))
        psav = ctx.enter_context(tc.tile_pool(name="psav", bufs=2, space="PSUM"))

        # ---- constants + weights to SBUF (ordered by first use) ----
        cos_s = cst.tile([128, S], BF16)
        sin_s = cst.tile([128, S], BF16)
        p2t_s = cst.tile([128, 128], BF16)
        id_s = cst.tile([128, 128], BF16)
        on_s = cst.tile([1, DK], BF16)
        wq_s = cst.tile([128, 8, 128], BF16)
        wk_s = cst.tile([128, 8, 128], BF16)
        wv_s = cst.tile([128, 8, 128], BF16)
        wo_s = cst.tile([128, D], BF16)
        def issue_xt(t):
            """Prefetch token tile t of x (feature-major)."""
            xt = stream.tile([128, 8, TT], BF16, tag="xt")
            for g in range(8):
                nc.sync.dma_start(
                    out=xt[:, g, :],
                    in_=xT[g * 128:(g + 1) * 128, t * TT:(t + 1) * TT],
                )
            return xt

        xt0 = issue_xt(0)          # before the const loads on the sync queue
        # first-needed consts on the scalar queue, the rest trail xt0 on sync
        nc.scalar.dma_start(
            out=wq_s[:], in_=wq.ap().rearrange("(g p) m -> p g m", p=128))
        nc.scalar.dma_start(out=sin_s[:], in_=c_sin[:, :])
        nc.scalar.dma_start(out=cos_s[:], in_=c_cos[:, :])
        nc.scalar.dma_start(out=p2t_s[:], in_=c_p2t[:, :])
        nc.sync.dma_start(
            out=wk_s[:], in_=wk.ap().rearrange("(g p) m -> p g m", p=128))
        nc.sync.dma_start(
            out=wv_s[:], in_=wv.ap().rearrange("(g p) m -> p g m", p=128))
        nc.sync.dma_start(out=id_s[:], in_=c_id[:, :])
        nc.sync.dma_start(out=on_s[:], in_=c_on[:, :])
        nc.sync.dma_start(out=wo_s[:], in_=wo.ap())

        # persistent activations
        qrot = persist.tile([128, T], BF16)
        krot = persist.tile([128, T], BF16)
        # per k-block: [v_h0(64) | ones | pad to 72 | v_h1(64) | ones | pad]
        v_sb = persist.tile([128, T // KT, 2 * VP], BF16)
        # cols 64/136 stay 1.0 (denominator ones)
        nc.vector.memset(v_sb[:].rearrange("p a b -> p (a b)"), 1.0)

        def phase1_qk(t, xt):
            """Project token tile t to q/k and rope them. Both projection
            groups are issued before the perm matmuls so the sin/cos muls of
            Q overlap the K projection on PE."""
            pos = slice((t % (S // TT)) * TT, (t % (S // TT)) * TT + TT)
            tok = slice(t * TT, (t + 1) * TT)
            p_qs, qss, qcs = [], [], []
            for w_sb in (wq_s, wk_s):
                p_q = ps.tile([128, TT], F32, tag="p1")
                for g in range(8):
                    nc.tensor.matmul(p_q[:], w_sb[:, g, :], xt[:, g, :],
                                     start=(g == 0), stop=(g == 7))
                p_qs.append(p_q)
                qs = tmp.tile([128, TT], BF16, tag="qs")
                nc.vector.tensor_mul(qs[:], p_q[:], sin_s[:, pos])
                qss.append(qs)
                qc = tmp.tile([128, TT], F32, tag="qc")
                nc.vector.tensor_mul(qc[:], p_q[:], cos_s[:, pos])
                qcs.append(qc)
            for k, dst in enumerate((qrot, krot)):
                p_perm = ps.tile([128, TT], F32, tag="p1")
                nc.tensor.matmul(p_perm[:], p2t_s[:], qss[k][:],
                                 start=True, stop=True)
                nc.vector.tensor_add(dst[:, tok], qcs[k][:], p_perm[:])

        def phase1_v(t, xt):
            """Project v for tile t and transpose to [k-pos, dk] layout."""
            p_v = ps.tile([128, TT], F32, tag="p1")
            for g in range(8):
                nc.tensor.matmul(p_v[:], wv_s[:, g, :], xt[:, g, :],
                                 start=(g == 0), stop=(g == 7))
            vt = tmp.tile([128, TT], BF16, tag="vt")
            nc.vector.tensor_copy(vt[:], p_v[:])
            p_t = ps.tile([128, 4, 128], BF16, tag="p1")
            for blk in range(TT // 128):
                nc.tensor.transpose(p_t[:, blk, :],
                                    vt[:, blk * 128:(blk + 1) * 128], id_s[:])
            g0 = t * (TT // 128)
            vdst = (v_sb[:, g0:g0 + 4, :]
                    .rearrange("p g (a c) -> p g a c", a=2)[:, :, :, 0:64])
            nc.vector.tensor_copy(
                vdst, p_t[:].rearrange("p g (a c) -> p g a c", a=2)
            )

        def make_attention(b, J):
            """q-tile J (512 wide) of batch b, both heads paired.

            Returns (scores_exp, av, finish) emitters so the caller can
            interleave score/exp of early blocks with phase1_v and keep a
            2-block exp lookahead ahead of the AV accumulation.
            """
            av0 = psav.tile([65, TT], F32, tag="av")
            av1 = psav.tile([65, TT], F32, tag="av")
            av = [av0, av1]
            nk = 4 * (J + 1)
            ats = {}

            def scores_exp(i):
                r = i - 4 * J          # >= 0 on diagonal blocks
                qo = KT * r if r > 0 else 0    # causal-narrowed q offset
                n = TT - qo
                p_s = ps.tile([128, 2, TT], F32, tag="mm")
                for h in range(2):
                    hp = slice(64 * h, 64 * h + 64)
                    nc.tensor.matmul(
                        p_s[:, h, 0:n],
                        krot[hp, b * S + i * KT: b * S + (i + 1) * KT],
                        qrot[hp, b * S + J * TT + qo: b * S + (J + 1) * TT],
                        start=True, stop=True,
                    )
                at = attnp.tile([128, 2, TT], BF16, tag="at")
                nc.scalar.activation(at[:, :, 0:n], p_s[:, :, 0:n], AF.Exp,
                                     scale=float(SCALE))
                if r >= 0:  # diagonal 128-block: zero where k > q
                    for h in range(2):
                        nc.gpsimd.affine_select(
                            out=at[:, h, 0:KT], in_=at[:, h, 0:KT],
                            compare_op=mybir.AluOpType.is_ge,
                            fill=0.0, base=0,
                            pattern=[[1, KT]], channel_multiplier=-1,
                        )
                ats[i] = at

            def av_acc(i):
                r = i - 4 * J
                qo = KT * r if r > 0 else 0
                n = TT - qo
                at = ats.pop(i)
                g = b * (S // KT) + i
                for h in range(2):
                    nc.tensor.matmul(
                        av[h][:, qo:TT],
                        v_sb[:, g, VP * h:VP * h + 65],
                        at[:, h, 0:n],
                        start=(i == 0), stop=(i == nk - 1),
                    )

            stage = outp.tile([128, TT], BF16, tag="stage")

            def finish_norm(cs=slice(0, TT)):
                # 1/denom on DVE straight from PSUM, replicate across 64
                # partitions with a ones-column matmul, then scale av.
                w = cs.stop - cs.start
                recs = []
                for h in range(2):
                    rec = tmp.tile([1, TT], BF16, tag="rec")
                    with nc.allow_low_precision(reason="bf16 recip"):
                        nc.vector.reciprocal(rec[:, 0:w], av[h][64:65, cs])
                    recs.append(rec)
                pbs = []
                for h in range(2):
                    pb = ps.tile([64, TT], F32, tag="p1")
                    nc.tensor.matmul(pb[:, 0:w], on_s[:], recs[h][:, 0:w],
                                     start=True, stop=True)
                    pbs.append(pb)
                for h in range(2):  # PSUM->SBUF hop (hw: one PSUM input/op)
                    hp = slice(64 * h, 64 * h + 64)
                    if h == 0:
                        nc.scalar.copy(stage[hp, cs], av[h][0:64, cs])
                    else:
                        nc.vector.tensor_copy(stage[hp, cs], av[h][0:64, cs])
                for h in range(2):
                    hp = slice(64 * h, 64 * h + 64)
                    nc.vector.tensor_mul(stage[hp, cs], stage[hp, cs],
                                         pbs[h][:, 0:w])

            def finish_proj(blks=range(TT // 128)):
                # output projection for this tile: po[tok,1024] = stage.T@Wo_c
                # PSUM->SBUF copies go mostly to Act when the NEXT tile's
                # attention is small (Act has slack there), else mostly DVE.
                t = 4 * b + J
                act_heavy = None  # 2/2 split below
                for blk in blks:
                    po = ps.tile([128, 2, TT], F32, tag="mm")
                    for hf in range(2):  # psum bank = 512 f32: 2 half-matmuls
                        nc.tensor.matmul(po[:, hf, :],
                                         stage[:, blk * 128:(blk + 1) * 128],
                                         wo_s[:, hf * TT:(hf + 1) * TT],
                                         start=True, stop=True)
                    yt = outp.tile([128, D], BF16, tag="yt")
                    pov = po[:].rearrange("p a b -> p (a b)")
                    on_act = blk % 2 == 0
                    if on_act:
                        nc.scalar.copy(yt[:], pov)
                    else:
                        nc.vector.tensor_copy(yt[:], pov)
                    deng = nc.sync if blk % 2 == 0 else nc.gpsimd
                    deng.dma_start(
                        out=y[t * TT + blk * 128: t * TT + (blk + 1) * 128, :],
                        in_=yt[:],
                    )

            return nk, scores_exp, av_acc, finish_norm, finish_proj

        # interleave: attention(b, J) depends only on token tiles <= t;
        # exp of blocks 0/1 cooks during phase1_v; xt(t+1) prefetches during
        # attention(t); AV trails scores/exp by 2 blocks. The previous tile's
        # normalize+outproj are deferred into the next tile's phase1 so PE
        # has projection work to chew while DVE computes reciprocals.
        for rep in range(repeat):
            xt = xt0 if rep == 0 else issue_xt(0)
            fin = None
            for t in range(8):
                phase1_qk(t, xt)
                if fin is not None:
                    fin[0]()           # norm(t-1): recip/bcast/mul
                nk, scores_exp, av_acc, f_norm, f_proj = \
                    make_attention(t // 4, t % 4)
                scores_exp(0)
                scores_exp(1)
                phase1_v(t, xt)
                if t < 7:
                    xt = issue_xt(t + 1)
                if fin is not None:
                    fin[1]()           # outproj(t-1)
                for i in range(2, nk):
                    scores_exp(i)
                    av_acc(i - 2)
                av_acc(nk - 2)
                av_acc(nk - 1)
                fin = (f_norm, f_proj)
            # last tile: column-split finish so the first po matmuls start
            # after only half the normalize chain
            fin[0](slice(0, TT // 2))
            fin[1](range(0, 2))
            fin[0](slice(TT // 2, TT))
            fin[1](range(2, 4))

    if not _SIM:            # hw encoding workaround; confuses sim race detector
        split_multi_waits(nc)
    return nc


def _get_runner(repeat=1):
    """Build + jit once; returns f(in_maps) -> list of per-core output dicts."""
    key = ("runner", repeat)
    if key in _cache:
        return _cache[key]
    import jax
    import jax.numpy as jnp
    from jax.sharding import Mesh, PartitionSpec
    from jax.experimental.shard_map import shard_map
    from concourse import bass2jax, mybir as _mybir

    nc = build_nc(repeat=repeat)
    bass2jax.install_neuronx_cc_hook()

    in_names, out_names, out_avals, zero_outs = [], [], [], []
    for alloc in nc.m.functions[0].allocations:
        if not isinstance(_mybir.MemoryLocationSet, type) or not isinstance(
            alloc, _mybir.MemoryLocationSet
        ):
            continue
        name = alloc.memorylocations[0].name
        if alloc.kind == "ExternalInput":
            if name != "partition_id":
                in_names.append(name)
        elif alloc.kind == "ExternalOutput":
            out_names.append(name)
            shape = tuple(alloc.tensor_shape)
            dtype = _mybir.dt.np(alloc.dtype)
            out_avals.append(jax.core.ShapedArray(shape, dtype))
            zero_outs.append(np.zeros(shape, dtype))
    n_params = len(in_names)
    has_pid = nc.partition_id_tensor is not None
    all_names = in_names + out_names + (["partition_id"] if has_pid else [])

    def _body(*args):
        operands = list(args)
        if has_pid:
            operands.append(bass2jax.partition_id_tensor())
        outs = bass2jax._bass_exec_p.bind(
            *operands,
            out_avals=tuple(out_avals),
            in_names=tuple(all_names),
            out_names=tuple(out_names),
            lowering_input_output_aliases=(),
            sim_require_finite=True,
            sim_require_nnan=True,
            nc=nc,
        )
        return tuple(outs)

    devices = jax.devices()[:N_CORES]
    mesh = Mesh(np.asarray(devices), ("core",))
    n_outs = len(out_names)
    sharded = jax.jit(
        shard_map(
            _body, mesh=mesh,
            in_specs=(PartitionSpec("core"),) * (n_params + n_outs),
            out_specs=(PartitionSpec("core"),) * n_outs,
            check_rep=False,
        ),
        donate_argnums=() if _SIM else tuple(range(n_params, n_params + n_outs)),
        keep_unused=True,
    )

    def make_bench(in_maps):
        from jax.sharding import NamedSharding
        sh = NamedSharding(mesh, PartitionSpec("core"))
        concat_in = [
            jax.device_put(
                np.concatenate([np.asarray(m[nm]) for m in in_maps], axis=0), sh)
            for nm in in_names
        ]
        zshapes = [(N_CORES * z.shape[0], *z.shape[1:]) for z in zero_outs]
        zdt = [z.dtype for z in zero_outs]
        mkz = jax.jit(
            lambda: tuple(jnp.zeros(s, d) for s, d in zip(zshapes, zdt)),
            out_shardings=tuple(sh for _ in zshapes),
        )

        def bench_once():
            zs = mkz()
            jax.block_until_ready(zs)
            t0 = __import__("time").perf_counter()
            out = sharded(*concat_in, *zs)
            jax.block_until_ready(out)
            return __import__("time").perf_counter() - t0

        return bench_once

    def run(in_maps):
        concat_in = [
            np.concatenate([np.asarray(m[nm]) for m in in_maps], axis=0)
            for nm in in_names
        ]
        concat_zeros = [
            np.zeros((N_CORES * z.shape[0], *z.shape[1:]), z.dtype)
            for z in zero_outs
        ]
        out_arrs = sharded(*concat_in, *concat_zeros)
        return [
            {nm: np.asarray(out_arrs[i]).reshape(N_CORES, *out_avals[i].shape)[c]
             for i, nm in enumerate(out_names)}
            for c in range(N_CORES)
        ]

    run.make_bench = make_bench
    _cache[key] = run
    return run


def _prep_in_maps(x, Wq, Wk, Wv, Wo):
    xT = np.ascontiguousarray(x.reshape(T, D).T).astype(NPBF)
    in_maps = []
    for c in range(N_CORES):
        in_maps.append({
            "xT": xT,
            "wq": np.ascontiguousarray(
                np.concatenate([Wq[2 * c], Wq[2 * c + 1]], 1)).astype(NPBF),
            "wk": np.ascontiguousarray(
                np.concatenate([Wk[2 * c], Wk[2 * c + 1]], 1)).astype(NPBF),
            "wv": np.ascontiguousarray(
                np.concatenate([Wv[2 * c], Wv[2 * c + 1]], 1)).astype(NPBF),
            "wo": np.ascontiguousarray(Wo[128 * c:128 * (c + 1), :]).astype(NPBF),
        })
    return in_maps


def kernel(x, Wq, Wk, Wv, Wo, repeat=1):
    x, Wq, Wk, Wv, Wo = (np.asarray(a, np.float32) for a in (x, Wq, Wk, Wv, Wo))
    run = _get_runner(repeat=repeat)
    results = run(_prep_in_maps(x, Wq, Wk, Wv, Wo))
    out = results[0]["y"].astype(np.float32)
    for r in results[1:]:
        out += r["y"].astype(np.float32)
    return out.reshape(B, S, D)


# revision 87
# speedup vs baseline: 1.0468x; 1.0048x over previous
"""Multi-head attention (RoPE, causal) Trainium2 Bass kernel, 8-core SPMD.

Sharding: tensor-parallel over heads (2 heads/core) for QKV+attention;
the output projection is contraction-parallel: each core multiplies its
head-pair's attention output by its 128-row slice of Wo and streams a
full-token partial y to DRAM; the 8-way reduction happens on the host.
This removes the AllToAll entirely (its cost model is 15us overhead +
2MB/40GBps ~= 67us of serial tail).

bf16 activations/weights throughout (inputs host-cast); PSUM stays f32.

Math per core c (heads h0=2c, h1=2c+1):
  qT = Wq2.T @ xT            (feature-on-partition layout throughout)
  rope: qrot = qT*cosT + P2@(qT*sinT)   (P2 = pair-swap-with-sign, const)
  scoresT[k,q] = krot.T @ qrot tiles    ([k-part, q-free] layout)
  attnT = exp(scoresT/8), causal-masked on diagonal tiles
  AV: out[65,q] = [v | ones].T @ attnT  (row 64 = softmax denominator)
  normalize, po[tok,1024] = stage.T @ Wo_c, stream to y_partial DRAM
"""
import os
import numpy as np
import ml_dtypes
from contextlib import ExitStack

import concourse.bass as bass
import concourse.mybir as mybir
import concourse.tile as tile
from concourse import library_config

_SIM = os.environ.get("BASS_SIM") == "1"   # CoreSim (CPU) iteration mode

N_CORES = 8
B, S, D, H, DK = 2, 2048, 1024, 16, 64
T = B * S                    # 4096 flat tokens, batch-major
TT = 512                     # token tile (phase 1 / q tiles)
KT = 128                     # k tile (scores partition dim)
NT = T // TT                 # 8 token tiles
F32 = mybir.dt.float32
BF16 = mybir.dt.bfloat16
F8E4 = mybir.dt.float8e4          # e4m3: v / x / wv (max 240)
F8E5 = mybir.dt.float8e5          # e5m2: exp(scores) (wide range)
AF = mybir.ActivationFunctionType
PM = mybir.MatmulPerfMode
SCALE = 1.0 / np.sqrt(DK)
EXPB = -3.0                       # exp bias, cancelled by the denominator
NPBF = ml_dtypes.bfloat16
NPF8 = ml_dtypes.float8_e4m3
VP = 72                           # v_sb per-head stride (64 v + 1 ones + pad)

_cache = {}


def _consts():
    inv_freq = 10000.0 ** (-(np.arange(0, DK, 2, dtype=np.float64) / DK))
    pos = np.arange(S, dtype=np.float64)
    ang = pos[:, None] * inv_freq[None, :]                 # [S, 32]
    cos = np.repeat(np.cos(ang), 2, axis=1).T              # [64, S]
    sin = np.repeat(np.sin(ang), 2, axis=1).T
    cosT = np.concatenate([cos, cos], 0).astype(NPBF)   # [128, S]
    sinT = np.concatenate([sin, sin], 0).astype(NPBF)
    # P2T = P.T blockdiag for 2 heads; (P v)[2i] = -v[2i+1], (P v)[2i+1] = v[2i]
    p = np.zeros((DK, DK), np.float32)
    for i in range(DK // 2):
        p[2 * i, 2 * i + 1] = -1.0
        p[2 * i + 1, 2 * i] = 1.0
    p2t = np.zeros((128, 128), np.float32)
    p2t[:DK, :DK] = p.T
    p2t[DK:, DK:] = p.T
    ident = np.eye(128, dtype=np.float32)
    ones64 = np.ones((1, DK), np.float32)
    return (cosT, sinT, p2t.astype(NPBF), ident.astype(NPBF),
            ones64.astype(NPBF))


def split_multi_waits(nc, max_waits=1):
    """This walrus build allows fewer sync-waits per instruction than Tile's
    final drain carries; hoist extras onto same-engine NOPs inserted before."""
    for fn in nc.m.functions:
        for blk in fn.blocks:
            insts = blk.instructions
            out = []
            for inst in insts:
                si = getattr(inst, "sync_info", None)
                waits = list(si.on_wait) if si is not None else []
                if len(waits) > max_waits:
                    extra, keep = waits[:-max_waits], waits[-max_waits:]
                    for j, w in enumerate(extra):
                        nop = mybir.InstNoOp(
                            name=f"{inst.name}-wsplit{j}", ins=[], outs=[]
                        )
                        nop.engine = inst.engine
                        nop.sync_info = mybir.SyncInfo(on_wait=[w], on_update=[])
                        out.append(nop)
                    inst.sync_info = mybir.SyncInfo(
                        on_wait=keep, on_update=list(si.on_update)
                    )
                out.append(inst)
            insts[:] = out


def build_nc(repeat=1):
    cosT_np, sinT_np, p2t_np, ident_np, ones64_np = _consts()

    nc = bass.Bass("TRN2", target_bir_lowering=False, debug=False,
                   num_devices=N_CORES)
    xT = nc.declare_dram_parameter("xT", [D, T], BF16, isOutput=False)
    wq = nc.declare_dram_parameter("wq", [D, 128], BF16, isOutput=False)
    wk = nc.declare_dram_parameter("wk", [D, 128], BF16, isOutput=False)
    wv = nc.declare_dram_parameter("wv", [D, 128], BF16, isOutput=False)
    wo = nc.declare_dram_parameter("wo", [128, D], BF16, isOutput=False)
    y = nc.declare_dram_parameter("y", [T, D], BF16, isOutput=True)

    dnorm = nc.dram_tensor("dnorm", [2, 2, 1, TT], BF16)   # 1/denom bounce
    c_cos = nc.inline_tensor(cosT_np, name="c_cos")
    c_sin = nc.inline_tensor(sinT_np, name="c_sin")
    c_p2t = nc.inline_tensor(p2t_np, name="c_p2t")
    c_id = nc.inline_tensor(ident_np, name="c_id")
    c_on = nc.inline_tensor(ones64_np, name="c_on")

    with tile.TileContext(nc) as tc, ExitStack() as ctx:
        cst = ctx.enter_context(tc.tile_pool(name="cst", bufs=1))
        stream = ctx.enter_context(tc.tile_pool(name="stream", bufs=2))
        persist = ctx.enter_context(tc.tile_pool(name="persist", bufs=1))
        tmp = ctx.enter_context(tc.tile_pool(name="tmp", bufs=3))
        attnp = ctx.enter_context(tc.tile_pool(name="attnp", bufs=4))
        outp = ctx.enter_context(tc.tile_pool(name="outp", bufs=4))
        ps = ctx.enter_context(tc.tile_pool(name="ps", bufs=2, space="PSUM"))
        psav = ctx.enter_context(tc.tile_pool(name="psav", bufs=2, space="PSUM"))

        # ---- constants + weights to SBUF (ordered by first use) ----
        cos_s = cst.tile([128, S], BF16)
        sin_s = cst.tile([128, S], BF16)
        p2t_s = cst.tile([128, 128], BF16)
        id_s = cst.tile([128, 128], BF16)
        on_s = cst.tile([1, DK], BF16)
        wq_s = cst.tile([128, 8, 128], BF16)
        wk_s = cst.tile([128, 8, 128], BF16)
        wv_s = cst.tile([128, 8, 128], BF16)
        wo_s = cst.tile([128, D], BF16)
        def issue_xt(t):
            """Prefetch token tile t of x (feature-major)."""
            xt = stream.tile([128, 8, TT], BF16, tag="xt")
            for g in range(8):
                nc.sync.dma_start(
                    out=xt[:, g, :],
                    in_=xT[g * 128:(g + 1) * 128, t * TT:(t + 1) * TT],
                )
            return xt

        xt0 = issue_xt(0)          # before the const loads on the sync queue
        # first-needed consts on the scalar queue, the rest trail xt0 on sync
        nc.scalar.dma_start(
            out=wq_s[:], in_=wq.ap().rearrange("(g p) m -> p g m", p=128))
        nc.scalar.dma_start(out=sin_s[:], in_=c_sin[:, :])
        nc.scalar.dma_start(out=cos_s[:], in_=c_cos[:, :])
        nc.scalar.dma_start(out=p2t_s[:], in_=c_p2t[:, :])
        nc.sync.dma_start(
            out=wk_s[:], in_=wk.ap().rearrange("(g p) m -> p g m", p=128))
        nc.sync.dma_start(
            out=wv_s[:], in_=wv.ap().rearrange("(g p) m -> p g m", p=128))
        nc.sync.dma_start(out=id_s[:], in_=c_id[:, :])
        nc.sync.dma_start(out=on_s[:], in_=c_on[:, :])
        nc.sync.dma_start(out=wo_s[:], in_=wo.ap())

        # persistent activations
        qrot = persist.tile([128, T], BF16)
        krot = persist.tile([128, T], BF16)
        # per k-block: [v_h0(64) | ones | pad to 72 | v_h1(64) | ones | pad]
        v_sb = persist.tile([128, T // KT, 2 * VP], BF16)
        # cols 64/136 stay 1.0 (denominator ones)
        nc.vector.memset(v_sb[:].rearrange("p a b -> p (a b)"), 1.0)

        def phase1_qk(t, xt):
            """Project token tile t to q/k and rope them. Both projection
            groups are issued before the perm matmuls so the sin/cos muls of
            Q overlap the K projection on PE."""
            pos = slice((t % (S // TT)) * TT, (t % (S // TT)) * TT + TT)
            tok = slice(t * TT, (t + 1) * TT)
            p_qs, qss, qcs = [], [], []
            for w_sb in (wq_s, wk_s):
                p_q = ps.tile([128, TT], F32, tag="p1")
                for g in range(8):
                    nc.tensor.matmul(p_q[:], w_sb[:, g, :], xt[:, g, :],
                                     start=(g == 0), stop=(g == 7))
                p_qs.append(p_q)
                qs = tmp.tile([128, TT], BF16, tag="qs")
                nc.vector.tensor_mul(qs[:], p_q[:], sin_s[:, pos])
                qss.append(qs)
                qc = tmp.tile([128, TT], F32, tag="qc")
                nc.vector.tensor_mul(qc[:], p_q[:], cos_s[:, pos])
                qcs.append(qc)
            def rope_fin(k, dst):
                p_perm = ps.tile([128, TT], F32, tag="p1")
                nc.tensor.matmul(p_perm[:], p2t_s[:], qss[k][:],
                                 start=True, stop=True)
                nc.vector.tensor_add(dst[:, tok], qcs[k][:], p_perm[:])

            rope_fin(0, qrot)
            return lambda: rope_fin(1, krot)   # K-rope, deferrable for J>0

        def phase1_v(t, xt):
            """Project v for tile t and transpose to [k-pos, dk] layout."""
            p_v = ps.tile([128, TT], F32, tag="p1")
            for g in range(8):
                nc.tensor.matmul(p_v[:], wv_s[:, g, :], xt[:, g, :],
                                 start=(g == 0), stop=(g == 7))
            vt = tmp.tile([128, TT], BF16, tag="vt")
            nc.vector.tensor_copy(vt[:], p_v[:])
            p_t = ps.tile([128, 4, 128], BF16, tag="p1")
            for blk in range(TT // 128):
                nc.tensor.transpose(p_t[:, blk, :],
                                    vt[:, blk * 128:(blk + 1) * 128], id_s[:])
            g0 = t * (TT // 128)
            vdst = (v_sb[:, g0:g0 + 4, :]
                    .rearrange("p g (a c) -> p g a c", a=2)[:, :, :, 0:64])
            nc.vector.tensor_copy(
                vdst, p_t[:].rearrange("p g (a c) -> p g a c", a=2)
            )

        def make_attention(b, J):
            """q-tile J (512 wide) of batch b, both heads paired.

            Returns (scores_exp, av, finish) emitters so the caller can
            interleave score/exp of early blocks with phase1_v and keep a
            2-block exp lookahead ahead of the AV accumulation.
            """
            av0 = psav.tile([65, TT], F32, tag="av")
            av1 = psav.tile([65, TT], F32, tag="av")
            av = [av0, av1]
            nk = 4 * (J + 1)
            ats = {}

            def scores_exp(i):
                r = i - 4 * J          # >= 0 on diagonal blocks
                qo = KT * r if r > 0 else 0    # causal-narrowed q offset
                n = TT - qo
                p_s = ps.tile([128, 2, TT], F32, tag="mm")
                for h in range(2):
                    hp = slice(64 * h, 64 * h + 64)
                    nc.tensor.matmul(
                        p_s[:, h, 0:n],
                        krot[hp, b * S + i * KT: b * S + (i + 1) * KT],
                        qrot[hp, b * S + J * TT + qo: b * S + (J + 1) * TT],
                        start=True, stop=True,
                    )
                at = attnp.tile([128, 2, TT], BF16, tag="at")
                nc.scalar.activation(at[:, :, 0:n], p_s[:, :, 0:n], AF.Exp,
                                     scale=float(SCALE))
                if r >= 0:  # diagonal 128-block: zero where k > q
                    for h in range(2):
                        nc.gpsimd.affine_select(
                            out=at[:, h, 0:KT], in_=at[:, h, 0:KT],
                            compare_op=mybir.AluOpType.is_ge,
                            fill=0.0, base=0,
                            pattern=[[1, KT]], channel_multiplier=-1,
                        )
                ats[i] = at

            def av_acc(i):
                r = i - 4 * J
                qo = KT * r if r > 0 else 0
                n = TT - qo
                at = ats.pop(i)
                g = b * (S // KT) + i
                for h in range(2):
                    nc.tensor.matmul(
                        av[h][:, qo:TT],
                        v_sb[:, g, VP * h:VP * h + 65],
                        at[:, h, 0:n],
                        start=(i == 0), stop=(i == nk - 1),
                    )

            stage = outp.tile([128, TT], BF16, tag="stage")

            def finish_norm(cs=slice(0, TT)):
                # 1/denom on DVE straight from PSUM, replicate across 64
                # partitions with a ones-column matmul, then scale av.
                w = cs.stop - cs.start
                recs = []
                for h in range(2):
                    rec = tmp.tile([1, TT], BF16, tag="rec")
                    with nc.allow_low_precision(reason="bf16 recip"):
                        nc.vector.reciprocal(rec[:, 0:w], av[h][64:65, cs])
                    recs.append(rec)
                pbs = []
                for h in range(2):
                    pb = ps.tile([64, TT], F32, tag="p1")
                    nc.tensor.matmul(pb[:, 0:w], on_s[:], recs[h][:, 0:w],
                                     start=True, stop=True)
                    pbs.append(pb)
                split = cs.stop - cs.start != TT   # last-tile halves
                for h in range(2):  # PSUM->SBUF hop (hw: one PSUM input/op)
                    hp = slice(64 * h, 64 * h + 64)
                    if h == 0 or split:
                        nc.scalar.copy(stage[hp, cs], av[h][0:64, cs])
                    else:
                        nc.vector.tensor_copy(stage[hp, cs], av[h][0:64, cs])
                for h in range(2):
                    hp = slice(64 * h, 64 * h + 64)
                    nc.vector.tensor_mul(stage[hp, cs], stage[hp, cs],
                                         pbs[h][:, 0:w])

            def finish_proj(blks=range(TT // 128)):
                # output projection for this tile: po[tok,1024] = stage.T@Wo_c
                # PSUM->SBUF copies go mostly to Act when the NEXT tile's
                # attention is small (Act has slack there), else mostly DVE.
                t = 4 * b + J
                act_heavy = None  # 2/2 split below
                for blk in blks:
                    po = ps.tile([128, 2, TT], F32, tag="mm")
                    for hf in range(2):  # psum bank = 512 f32: 2 half-matmuls
                        nc.tensor.matmul(po[:, hf, :],
                                         stage[:, blk * 128:(blk + 1) * 128],
                                         wo_s[:, hf * TT:(hf + 1) * TT],
                                         start=True, stop=True)
                    yt = outp.tile([128, D], BF16, tag="yt")
                    pov = po[:].rearrange("p a b -> p (a b)")
                    on_act = blk % 2 == 0
                    if on_act:
                        nc.scalar.copy(yt[:], pov)
                    else:
                        nc.vector.tensor_copy(yt[:], pov)
                    deng = nc.sync if blk % 2 == 0 else nc.gpsimd
                    deng.dma_start(
                        out=y[t * TT + blk * 128: t * TT + (blk + 1) * 128, :],
                        in_=yt[:],
                    )

            return nk, scores_exp, av_acc, finish_norm, finish_proj

        # interleave: attention(b, J) depends only on token tiles <= t;
        # exp of blocks 0/1 cooks during phase1_v; xt(t+1) prefetches during
        # attention(t); AV trails scores/exp by 2 blocks. The previous tile's
        # normalize+outproj are deferred into the next tile's phase1 so PE
        # has projection work to chew while DVE computes reciprocals.
        for rep in range(repeat):
            xt = xt0 if rep == 0 else issue_xt(0)
            fin = None
            for t in range(8):
                rope_k = phase1_qk(t, xt)
                if t % 4 == 0:
                    rope_k()           # J=0: first score blocks ARE diagonal
                nk, scores_exp, av_acc, f_norm, f_proj = \
                    make_attention(t // 4, t % 4)
                scores_exp(0)
                scores_exp(1)
                if t % 4 != 0:
                    rope_k()           # J>0: blocks 0/1 use old tiles' krot
                if fin is not None:
                    fin[0]()           # norm(t-1): recip/bcast/mul
                phase1_v(t, xt)
                if t < 7:
                    xt = issue_xt(t + 1)
                if fin is not None:
                    fin[1]()           # outproj(t-1)
                for i in range(2, nk):
                    scores_exp(i)
                    av_acc(i - 2)
                av_acc(nk - 2)
                av_acc(nk - 1)
                fin = (f_norm, f_proj)
            # last tile: column-split finish so the first po matmuls start
            # after only half the normalize chain
            fin[0](slice(0, TT // 2))
            fin[1](range(0, 2))
            fin[0](slice(TT // 2, TT))
            fin[1](range(2, 4))

    if not _SIM:            # hw encoding workaround; confuses sim race detector
        split_multi_waits(nc)
    return nc


def _get_runner(repeat=1):
    """Build + jit once; returns f(in_maps) -> list of per-core output dicts."""
    key = ("runner", repeat)
    if key in _cache:
        return _cache[key]
    import jax
    import jax.numpy as jnp
    from jax.sharding import Mesh, PartitionSpec
    from jax.experimental.shard_map import shard_map
    from concourse import bass2jax, mybir as _mybir

    nc = build_nc(repeat=repeat)
    bass2jax.install_neuronx_cc_hook()

    in_names, out_names, out_avals, zero_outs = [], [], [], []
    for alloc in nc.m.functions[0].allocations:
        if not isinstance(_mybir.MemoryLocationSet, type) or not isinstance(
            alloc, _mybir.MemoryLocationSet
        ):
            continue
        name = alloc.memorylocations[0].name
        if alloc.kind == "ExternalInput":
            if name != "partition_id":
                in_names.append(name)
        elif alloc.kind == "ExternalOutput":
            out_names.append(name)
            shape = tuple(alloc.tensor_shape)
            dtype = _mybir.dt.np(alloc.dtype)
            out_avals.append(jax.core.ShapedArray(shape, dtype))
            zero_outs.append(np.zeros(shape, dtype))
    n_params = len(in_names)
    has_pid = nc.partition_id_tensor is not None
    all_names = in_names + out_names + (["partition_id"] if has_pid else [])

    def _body(*args):
        operands = list(args)
        if has_pid:
            operands.append(bass2jax.partition_id_tensor())
        outs = bass2jax._bass_exec_p.bind(
            *operands,
            out_avals=tuple(out_avals),
            in_names=tuple(all_names),
            out_names=tuple(out_names),
            lowering_input_output_aliases=(),
            sim_require_finite=True,
            sim_require_nnan=True,
            nc=nc,
        )
        return tuple(outs)

    devices = jax.devices()[:N_CORES]
    mesh = Mesh(np.asarray(devices), ("core",))
    n_outs = len(out_names)
    sharded = jax.jit(
        shard_map(
            _body, mesh=mesh,
            in_specs=(PartitionSpec("core"),) * (n_params + n_outs),
            out_specs=(PartitionSpec("core"),) * n_outs,
            check_rep=False,
        ),
        donate_argnums=() if _SIM else tuple(range(n_params, n_params + n_outs)),
        keep_unused=True,
    )

    def make_bench(in_maps):
        from jax.sharding import NamedSharding
        sh = NamedSharding(mesh, PartitionSpec("core"))
        concat_in = [
            jax.device_put(
                np.concatenate([np.asarray(m[nm]) for m in in_maps], axis=0), sh)
            for nm in in_names
        ]
        zshapes = [(N_CORES * z.shape[0], *z.shape[1:]) for z in zero_outs]
        zdt = [z.dtype for z in zero_outs]
        mkz = jax.jit(
            lambda: tuple(jnp.zeros(s, d) for s, d in zip(zshapes, zdt)),
            out_shardings=tuple(sh for _ in zshapes),
        )

        def bench_once():
            zs = mkz()
            jax.block_until_ready(zs)
            t0 = __import__("time").perf_counter()
            out = sharded(*concat_in, *zs)
            jax.block_until_ready(out)
            return __import__("time").perf_counter() - t0

        return bench_once

    def run(in_maps):
        concat_in = [
            np.concatenate([np.asarray(m[nm]) for m in in_maps], axis=0)
            for nm in in_names
        ]
        concat_zeros = [
            np.zeros((N_CORES * z.shape[0], *z.shape[1:]), z.dtype)
            for z in zero_outs
        ]
        out_arrs = sharded(*concat_in, *concat_zeros)
        return [
            {nm: np.asarray(out_arrs[i]).reshape(N_CORES, *out_avals[i].shape)[c]
             for i, nm in enumerate(out_names)}
            for c in range(N_CORES)
        ]

    run.make_bench = make_bench
    _cache[key] = run
    return run


def _prep_in_maps(x, Wq, Wk, Wv, Wo):
    xT = np.ascontiguousarray(x.reshape(T, D).T).astype(NPBF)
    in_maps = []
    for c in range(N_CORES):
        in_maps.append({
            "xT": xT,
            "wq": np.ascontiguousarray(
                np.concatenate([Wq[2 * c], Wq[2 * c + 1]], 1)).astype(NPBF),
            "wk": np.ascontiguousarray(
                np.concatenate([Wk[2 * c], Wk[2 * c + 1]], 1)).astype(NPBF),
            "wv": np.ascontiguousarray(
                np.concatenate([Wv[2 * c], Wv[2 * c + 1]], 1)).astype(NPBF),
            "wo": np.ascontiguousarray(Wo[128 * c:128 * (c + 1), :]).astype(NPBF),
        })
    return in_maps


def kernel(x, Wq, Wk, Wv, Wo, repeat=1):
    x, Wq, Wk, Wv, Wo = (np.asarray(a, np.float32) for a in (x, Wq, Wk, Wv, Wo))
    run = _get_runner(repeat=repeat)
    results = run(_prep_in_maps(x, Wq, Wk, Wv, Wo))
    out = results[0]["y"].astype(np.float32)
    for r in results[1:]:
        out += r["y"].astype(np.float32)
    return out.reshape(B, S, D)
